# revision 9
# baseline (speedup 1.0000x reference)
"""GumbelQuantizer forward on 8 Trainium2 NeuronCores.

Strategy (data-parallel over the bs*l token axis, per the sharding hint):
  - 32768 tokens are split into 8 shards of 4096 tokens; each core runs an
    identical Bass/Tile program on its shard. Weights + codebook replicated.
  - Per core:  hT = gelu(W1.T @ xT + b1)   (PE, f32r full-rate matmuls)
               logits = hT.T @ W2          (PE, token-major output)
               z = logits + (gumbels + b2) (DVE add; b2 pre-folded into
                                            gumbels on host)
               idx = argmax(z) per group   (DVE max/max_index)
               out = emb[idx]              (indirect-DMA gather per (t,g))
  - The straight-through estimator's forward value is hard one-hot exactly,
    so softmax is skipped; output is the gathered codebook row (exact fp32).
  - PE is the bottleneck (~139 us of f32r rows at ~2.2 GHz sustained); the
    schedule keeps it stall-free against the chip-HBM-limited startup burst
    (8 cores pull x0+W1+W2+gumbels at t=0; per-core share is ~200-340 B/ns):
      * mm1 only needs x + W1, so mm2 lags DEPTH=3 chunks behind: by the
        time mm2(0) issues (~40us, after the measured ~11us of HBM-bound
        mm1 rampup) W2 and gumbel chunk 0 have streamed in behind the
        critical x/W1 set.
      * the tail is tapered to FOUR 256-token chunks so the final mm2-only
        drain is six small tiles and the DVE/gather epilogue backlog at the
        end stays ~3us (256 is the f32r full-rate minimum moving dim).
      * queue split: sync (SP) carries x and gumbel chunk streams; gpsimd
        carries b1, W1 blocks, W2 quarters, then the codebook gathers on
        SWDGE; the scalar (Act) queue carries ONLY output stores so the
        Gelu ACT-table load and the gelus are never blocked behind DMA
        issues (this cost ~30us in the original schedule).  A dummy gelu at
        the top hoists the ACT table load to t~7us.
      * warmup matmuls ramp the PE p-state until x0/W1 land.
  - Codebook gathers stay per (tile, group) on the gpsimd SWDGE queue
    (~1.1 us/launch, 64 launches) which hides under PE.  (Batched
    multi-offset indirect DMA reads garbage on HW beyond the first offset
    per partition - verified - so 128-row gathers are the usable primitive.)
"""

import os
import sys

sys.path.insert(0, "/opt/trn_rl_repo")

import numpy as np

NCORES = 8
BS, L, DIM = 16, 2048, 512
NTOK = BS * L              # 32768 tokens total
TOK = NTOK // NCORES       # 4096 tokens per core
INNER = 1024
CODES = 320
G = 2
VD = 128                   # codebook row dim
KT1 = DIM // 128           # 4  k-tiles for mm1
IT = INNER // 128          # 8  inner tiles
NWARM = 12                 # PE p-state warmup matmuls
DEPTH = 3                  # chunks of software-pipeline lag between mm1/mm2

# chunk schedule: (token offset, token count); tail tapered to 256 tokens
CHUNKS = [(i * 512, 512) for i in range(6)] + \
         [(3072 + i * 256, 256) for i in range(4)]
NCH = len(CHUNKS)

_CACHE = {}


def _round_f32r(a: np.ndarray) -> np.ndarray:
    """Round fp32 values to the f32r grid (drop 12 mantissa bits, RNE)."""
    u = np.ascontiguousarray(a, np.float32).view(np.uint32).copy()
    low = u & 0xFFF
    keep = u & np.uint32(0xFFFFF000)
    round_up = (low > 0x800) | ((low == 0x800) & (((u >> 12) & 1) == 1))
    keep = keep + (round_up.astype(np.uint32) << 12)
    return keep.view(np.float32)


def _build_nc():
    import concourse.bass as bass
    import concourse.tile as tile
    from concourse import bacc, mybir

    f32 = mybir.dt.float32
    f32r = mybir.dt.float32r
    u32 = mybir.dt.uint32
    ADD = mybir.AluOpType.add
    GELU = mybir.ActivationFunctionType.Gelu

    nc = bacc.Bacc("TRN2")
    xT = nc.dram_tensor("xT", [DIM, TOK], f32r, kind="ExternalInput")
    gum = nc.dram_tensor("gum", [TOK * G, CODES], f32, kind="ExternalInput")
    W1 = nc.dram_tensor("W1", [DIM, INNER], f32r, kind="ExternalInput")
    W2 = nc.dram_tensor("W2", [INNER, G * CODES], f32r, kind="ExternalInput")
    b1 = nc.dram_tensor("b1", [INNER], f32, kind="ExternalInput")
    emb = nc.dram_tensor("emb", [CODES, VD], f32, kind="ExternalInput")
    out = nc.dram_tensor("out", [TOK, G * VD], f32, kind="ExternalOutput")

    with tile.TileContext(nc) as tc:
        with (
            tc.tile_pool(name="consts", bufs=1) as consts,
            tc.tile_pool(name="xp", bufs=3) as xp,
            tc.tile_pool(name="hp", bufs=DEPTH + 1) as hp,
            tc.tile_pool(name="gp", bufs=4) as gp,
            tc.tile_pool(name="zp", bufs=3) as zp,
            tc.tile_pool(name="m8p", bufs=4) as m8p,
            tc.tile_pool(name="mip", bufs=3) as mip,
            tc.tile_pool(name="op", bufs=3) as op,
            tc.tile_pool(name="ps1", bufs=3, space="PSUM") as ps1,
            tc.tile_pool(name="ps2", bufs=2, space="PSUM") as ps2,
        ):
            warm = consts.tile([128, 512], f32r)
            scr = consts.tile([128, 1], f32)
            # W1 as one tile per 128-column block: mm1's i-th accumulation
            # only waits for block i's DMA, not the whole 2MB of W1
            w1sb = [consts.tile([128, KT1, 4 * 128], f32r, name=f"w1h{i}")
                    for i in range(2)]
            w2sb = consts.tile([128, IT, G * CODES], f32r)
            b1sb = consts.tile([128, IT], f32)

            xTr = xT.rearrange("(k p) t -> p k t", p=128)
            W1r = W1.rearrange("(k p) i -> p k i", p=128)
            W2r = W2.rearrange("(k p) c -> p k c", p=128)
            # gumbels viewed per 128-token tile: [p, tile, g, code]
            gv = gum.rearrange("(tt p g) x -> p tt g x", p=128, g=G)
            # output viewed per 128-token tile: [p, tile, g, vd]
            ov = out.rearrange("(tt p) (g v) -> p tt g v", p=128, g=G)

            xsb = {}
            gsb = {}

            def issue_x(ci, eng):
                off, n = CHUNKS[ci]
                t = xp.tile([128, KT1, n], f32r, name="xt")
                xsb[ci] = t
                eng.dma_start(t[:], xTr[:, :, off:off + n])

            def issue_g(ci, eng):
                off, n = CHUNKS[ci]
                t0, ntt = off // 128, n // 128
                t = gp.tile([128, ntt, G, CODES], f32, name="gt")
                gsb[ci] = t
                eng.dma_start(t[:], gv[:, t0:t0 + ntt])

            # ---- prologue ------------------------------------------------
            # dummy gelu first: hoists the ACT (Gelu) table load to t~7us on
            # an otherwise-idle scalar engine (no loads ride the Act queue)
            nc.gpsimd.memset(warm[:].bitcast(u32), 0)
            nc.scalar.activation(scr[:], warm[:, 0:1].bitcast(f32), GELU)

            # startup loads (FIFO per queue = priority): sync: x0,x1,x2,g0;
            # gpsimd: b1, W1 blocks, W2 quarters
            issue_x(0, nc.sync)
            nc.gpsimd.dma_start(b1sb[:], b1.rearrange("(i p) -> p i", p=128))
            for hh in range(2):
                nc.gpsimd.dma_start(w1sb[hh][:],
                                    W1r[:, :, hh * 512:(hh + 1) * 512])
            issue_x(1, nc.sync)
            issue_x(2, nc.sync)
            issue_g(0, nc.sync)
            for q in range(4):
                nc.gpsimd.dma_start(
                    w2sb[:, :, q * 160:(q + 1) * 160],
                    W2r[:, :, q * 160:(q + 1) * 160])

            # warmup matmuls: ramp the PE p-state while the prologue DMAs run
            for _ in range(NWARM):
                ph = ps1.tile([128, 512], f32, name="ph")
                nc.tensor.matmul(ph[:], warm[:, 0:128], warm[:],
                                 start=True, stop=True)

            hsb = {}
            for c in range(NCH + DEPTH):
                # prefetch future chunks' inputs on the SP queue (pool bufs
                # gate how far ahead these can run)
                if c + 3 < NCH:
                    issue_x(c + 3, nc.sync)
                if 1 <= c < NCH:
                    issue_g(c, nc.sync)

                if c < NCH:
                    ntok = CHUNKS[c][1]
                    xs = xsb.pop(c)
                    hs = hp.tile([128, IT, ntok], f32r, name="ht")
                    hsb[c] = hs
                    for i in range(IT):
                        ph = ps1.tile([128, 512], f32, name="ph")
                        for k in range(KT1):
                            nc.tensor.matmul(
                                ph[:, 0:ntok],
                                w1sb[i // 4][:, k, (i % 4) * 128:
                                             (i % 4) * 128 + 128],
                                xs[:, k, :],
                                start=(k == 0),
                                stop=(k == KT1 - 1),
                            )
                        nc.scalar.activation(hs[:, i, :], ph[:, 0:ntok], GELU,
                                             bias=b1sb[:, i:i + 1])

                if c >= DEPTH:
                    cc = c - DEPTH
                    off, ntok = CHUNKS[cc]
                    ntt = ntok // 128
                    tt0 = off // 128
                    last = cc == NCH - 1
                    hs2 = hsb.pop(cc)
                    gs = gsb.pop(cc)
                    osb = op.tile([128, 4 * G, VD], f32, name="ot")
                    mi = mip.tile([128, 4 * G, 8], u32, name="mt")
                    for t in range(ntt):
                        pz = ps2.tile([128, G, 512], f32, name="pz")
                        for g2 in range(G):
                            for k in range(IT):
                                nc.tensor.matmul(
                                    pz[:, g2, 0:CODES],
                                    hs2[:, k, t * 128:(t + 1) * 128],
                                    w2sb[:, k, g2 * CODES:(g2 + 1) * CODES],
                                    start=(k == 0),
                                    stop=(k == IT - 1),
                                )
                        zsb = zp.tile([128, G, CODES], f32, name="zt")
                        nc.vector.tensor_tensor(zsb[:], pz[:, :, 0:CODES],
                                                gs[:, t], op=ADD)
                        for g2 in range(G):
                            m8 = m8p.tile([128, 8], f32, name="m8")
                            nc.vector.max(m8[:], zsb[:, g2, :])
                            nc.vector.max_index(mi[:, t * G + g2, :], m8[:],
                                                zsb[:, g2, :])
                            nc.gpsimd.indirect_dma_start(
                                out=osb[:, t * G + g2, :],
                                out_offset=None,
                                in_=emb[:],
                                in_offset=bass.IndirectOffsetOnAxis(
                                    ap=mi[:, t * G + g2, 0:1], axis=0),
                            )
                        if last:
                            # per-tile store keeps the final chain short
                            nc.scalar.dma_start(
                                ov[:, tt0 + t:tt0 + t + 1],
                                osb[:, t * G:(t + 1) * G, :])
                    if not last:
                        nc.scalar.dma_start(
                            ov[:, tt0:tt0 + ntt],
                            osb[:, 0:ntt * G, :])

    nc.compile()
    return nc


def kernel(**inputs) -> np.ndarray:
    from concourse.bass_utils import run_bass_kernel_spmd

    x = np.asarray(inputs["x"], np.float32)
    gumbels = np.asarray(inputs["gumbels"], np.float32)
    W1 = np.asarray(inputs["W1"], np.float32)
    b1 = np.asarray(inputs["b1"], np.float32)
    W2 = np.asarray(inputs["W2"], np.float32)
    b2 = np.asarray(inputs["b2"], np.float32)
    emb = np.asarray(inputs["emb"], np.float32)

    if "nc" not in _CACHE:
        _CACHE["nc"] = _build_nc()
    nc = _CACHE["nc"]

    xt = x.reshape(NTOK, DIM)
    W1r = _round_f32r(W1)
    W2r = _round_f32r(W2)
    # fold b2 into the gumbel noise: z = logits + b2 + gumbels
    gumb = gumbels.reshape(NTOK, G, CODES) + b2.reshape(G, CODES)
    gumb = gumb.reshape(NTOK * G, CODES)

    in_maps = []
    for c in range(NCORES):
        xT_c = _round_f32r(np.ascontiguousarray(xt[c * TOK:(c + 1) * TOK, :].T))
        in_maps.append({
            "xT": xT_c,
            "gum": np.ascontiguousarray(gumb[c * TOK * G:(c + 1) * TOK * G]),
            "W1": W1r,
            "W2": W2r,
            "b1": b1,
            "emb": emb,
        })

    trace = bool(int(os.environ.get("KERNEL_TRACE", "0")))
    res = run_bass_kernel_spmd(nc, in_maps, core_ids=list(range(NCORES)),
                               trace=trace)
    _CACHE["last_result"] = res
    outs = [res.results[c]["out"] for c in range(NCORES)]
    return np.concatenate(outs, axis=0).reshape(BS, L, G * VD)
